# revision 22
# baseline (speedup 1.0000x reference)
"""AttentionRNN Trainium2 kernel.

Data-parallel across 8 NeuronCores on the batch axis (B=8 -> 1 sequence per
core). Everything (embedding gather, input projection, sequential RNN scan,
additive attention, output projection) runs on-device; the host only reshapes
and shards inputs and reassembles the output.

v2: single-ACT scan step. x_proj is pre-folded into the scan PSUM block via a
matmul (32 steps per PSUM block), so each scan step is 4 tiny whh matmuls +
one [128,2] tanh. Attention energy is emitted in 2-row pieces spread across
scan steps so no single ACT instruction stalls the scan chain. The output
projection interleaves mt=0 (ready early) ahead of mt=1.

Self-contained: hardcodes all shapes; reads nothing from disk.
"""

import sys

sys.path.insert(0, "/opt/trn_rl_repo")

import numpy as np

import concourse.bacc as bacc
import concourse.mybir as mybir
import concourse.tile as tile
from concourse.bass import IndirectOffsetOnAxis
from concourse.bass_utils import run_bass_kernel_spmd
from concourse.masks import make_identity

V, E, H, B, T = 32000, 256, 256, 8, 256
P = 128
NCORE = 8
F32 = mybir.dt.float32
I32 = mybir.dt.int32
AF = mybir.ActivationFunctionType
BF16 = mybir.dt.bfloat16
FP16 = mybir.dt.float16
SCAN_NP = np.float16  # numpy dtype fed to the whh input
CH = 8  # t-chunk size in the attention energy loop
NS = 1000  # free-dim slab for the output projection stream
SUB = 500  # matmul free-dim sub-chunk (one PSUM bank)
NPRE = 15  # W_out slabs prefetched from kernel start
BLK = 64  # scan PSUM block (steps per x-prefold)
MT_LAG = 2  # how many slabs mt=1 trails mt=0 in the out-proj


def _r2(w):
    """[2P, M] -> [P, 2, M] with w2[p, k, m] = w[k*P+p, m]"""
    return np.ascontiguousarray(w.reshape(2, P, -1).transpose(1, 0, 2))


def _col(b):
    """[2P] -> [P, 2] with c[p, k] = b[k*P+p]"""
    return np.ascontiguousarray(b.reshape(2, P).T)


def build_nc(dbg=False, zb=False):
    nc = bacc.Bacc("TRN2", target_bir_lowering=False, debug=False)

    idx_d = nc.dram_tensor("idx", [P, 2], I32, kind="ExternalInput")
    emb_d = nc.dram_tensor("emb", [V, E], F32, kind="ExternalInput")
    wih_d = nc.dram_tensor("wih", [P, 2, H], F32, kind="ExternalInput")
    whh_d = nc.dram_tensor("whh", [P, 2, H], FP16, kind="ExternalInput")
    wac_d = nc.dram_tensor("wac", [P, 2, H], FP16, kind="ExternalInput")
    wap_d = nc.dram_tensor("wap", [P, 2, H], FP16, kind="ExternalInput")
    bxrow_d = nc.dram_tensor("bxrow", [1, 2 * H], F32, kind="ExternalInput")
    bac_d = nc.dram_tensor("bac", [P, 2], F32, kind="ExternalInput")
    bap_d = nc.dram_tensor("bap", [P, 2], F32, kind="ExternalInput")
    v_d = nc.dram_tensor("vcol", [P, 2], FP16, kind="ExternalInput")
    mask_d = nc.dram_tensor("maskadd", [P, 2, T], F32, kind="ExternalInput")
    wout_d = nc.dram_tensor("wout", [P, 4, V], BF16, kind="ExternalInput")
    bout_d = nc.dram_tensor("bout", [1, V], BF16, kind="ExternalInput")
    ones_d = nc.dram_tensor("ones", [1, P], BF16, kind="ExternalInput")
    out_d = nc.dram_tensor("out", [T, V], FP16, kind="ExternalOutput")
    if dbg:
        dbg_scores = nc.dram_tensor("dbg_scores", [P, 2, T], F32, kind="ExternalOutput")
        dbg_comb = nc.dram_tensor("dbg_comb", [P, 4, T], FP16, kind="ExternalOutput")
        dbg_xrow = nc.dram_tensor("dbg_xrow", [P, 2, H], FP16, kind="ExternalOutput")

    with tile.TileContext(nc) as tc:
        with tc.tile_pool(name="persist", bufs=1) as pp:
            # --- persistent SBUF state ---
            idx_sb = pp.tile([P, 2], I32)
            wih = pp.tile([P, 2, H], F32)
            whh = pp.tile([P, 2, H], FP16)
            wac = pp.tile([P, 2, H], FP16)
            wap = pp.tile([P, 2, H], FP16)
            bxrow = pp.tile([1, 2 * H], F32)
            bac = pp.tile([P, 2], F32)
            bap = pp.tile([P, 2], F32)
            vcol = pp.tile([P, 2], FP16)
            maskadd = pp.tile([P, 2, T], F32)
            ident = pp.tile([P, P], F32)
            ident16 = pp.tile([P, P], FP16)
            ones_row = pp.tile([1, P], BF16)
            ones32 = pp.tile([1, P], F32)
            embT = pp.tile([P, 2, T], F32)  # [e_p, et, t]
            xrow = pp.tile([P, 2, H], FP16)  # [t_p, tc, h] = x_proj rows
            combT = pp.tile([P, 4, T], FP16)  # [:,0:2]=context^T, [:,2:4]=Hs^T
            qT = pp.tile([P, 2, T], F32)
            kTb = pp.tile([P, 2, T], FP16)  # fp16 copy of kT for the energy adds
            scores = pp.tile([P, 2, T], F32)  # [tp, tc, j], t = tc*128+tp
            ssum = pp.tile([P, 2], F32)
            srecip = pp.tile([P, 2], F32)
            alphaT = pp.tile([P, 2, T], FP16)  # [j_p, jt, t]
            hs = pp.tile([P, 2, H], FP16)  # [t_p, tc, h] (Hs, untransposed)
            combTr = pp.tile([P, 4, T], BF16)  # bf16 copy for the out-proj

            nc.sync.dma_start(idx_sb[:], idx_d[:])
            nc.sync.dma_start(wih[:], wih_d[:])
            nc.sync.dma_start(whh[:], whh_d[:])
            nc.sync.dma_start(wac[:], wac_d[:])
            nc.sync.dma_start(wap[:], wap_d[:])
            nc.sync.dma_start(bxrow[:], bxrow_d[:])
            nc.sync.dma_start(bac[:], bac_d[:])
            nc.sync.dma_start(bap[:], bap_d[:])
            nc.sync.dma_start(vcol[:], v_d[:])
            nc.sync.dma_start(maskadd[:], mask_d[:])
            make_identity(nc, ident[:])
            nc.vector.tensor_copy(ident16[:], ident[:])
            nc.sync.dma_start(ones_row[:], ones_d[:])
            nc.gpsimd.memset(ones32[:], 1.0)
            nc.gpsimd.memset(hs[:], 0.0)

            # W_out slab prefetch pool: created first so slab DMAs stream
            # during the scan/attention phases.
            wp_cm = tc.tile_pool(name="wpool", bufs=NPRE)
            wp = wp_cm.__enter__()
            wsl_tiles = {}
            for s_ in range(NPRE):
                wsl = wp.tile([P, 4, NS], BF16, tag="wslab")
                nc.sync.dma_start(wsl[:], wout_d[:, :, s_ * NS : (s_ + 1) * NS])
                wsl_tiles[s_] = wsl

            # out-proj PSUM pool opened early so its 2 banks are reserved
            # (no aliasing stalls at the phase boundary).
            pg_cm = tc.tile_pool(name="pg_ps", bufs=2, space="PSUM")
            pg_ps = pg_cm.__enter__()
            op_cm = tc.tile_pool(name="opool", bufs=3)
            op = op_cm.__enter__()
            bp_cm = tc.tile_pool(name="bpool", bufs=2)
            bp = bp_cm.__enter__()

            # --- phase A: embedding gather + transpose to embT [e, t] ---
            with (
                tc.tile_pool(name="pha", bufs=2) as pa,
                tc.tile_pool(name="pha_ps", bufs=2, space="PSUM") as pa_ps,
            ):
                for c in range(2):
                    emb_g = pa.tile([P, E], F32, tag="embg")
                    nc.gpsimd.indirect_dma_start(
                        out=emb_g[:],
                        out_offset=None,
                        in_=emb_d[:, :],
                        in_offset=IndirectOffsetOnAxis(ap=idx_sb[:, c : c + 1], axis=0),
                    )
                    for et in range(2):
                        tr_ps = pa_ps.tile([P, P], F32, tag="trps")
                        nc.tensor.transpose(
                            tr_ps[:], emb_g[:, et * P : (et + 1) * P], ident[:]
                        )
                        nc.vector.tensor_copy(
                            embT[:, et, c * P : (c + 1) * P], tr_ps[:]
                        )

                # --- phase B: xrow[t, h] = emb @ W_ih (+ b_ih + b_hh) ---
                for tcn in range(2):
                    psx = pa_ps.tile([P, H], F32, tag="projps")
                    for et in range(2):
                        nc.tensor.matmul(
                            psx[:],
                            embT[:, et, tcn * P : (tcn + 1) * P],
                            wih[:, et, :],
                            start=(et == 0),
                            stop=False,
                            skip_group_check=True,
                        )
                    nc.tensor.matmul(
                        psx[:],
                        ones32[:],
                        bxrow[:, 0:H],
                        start=False,
                        stop=True,
                        skip_group_check=True,
                    )
                    nc.vector.tensor_copy(xrow[:, tcn, :], psx[:])
                if dbg:
                    nc.sync.dma_start(dbg_xrow[:], xrow[:])

            # --- phases C+D+E fused: scan with attention pipelined under it ---
            n_chunks = T // CH
            with (
                tc.tile_pool(name="scan_ps", bufs=2, space="PSUM") as sc_ps,
                tc.tile_pool(name="qk_ps", bufs=2, space="PSUM") as qk_ps,
                tc.tile_pool(name="epool", bufs=4) as ep,
                tc.tile_pool(name="rowpool", bufs=2) as rp,
                tc.tile_pool(name="scrpool", bufs=4, space="DRAM") as scrp,
                tc.tile_pool(name="eps", bufs=2, space="PSUM") as e_ps,
            ):
                def emit_xmm(ps_t, t, mt, stop=False):
                    """x_t into PSUM column (start=True) — resets the column.
                    MUST be immediately followed (in program order) by that
                    same (mt, column)'s whh matmuls: PSUM accumulation groups
                    are per-bank, and a later start=True begins a new group
                    whose first write to an address OVERWRITES it."""
                    t0 = (t // BLK) * BLK
                    tcn, t0p = t0 // P, t0 % P
                    ti = t - t0
                    nc.tensor.matmul(
                        ps_t[:, mt : mt + 1],
                        xrow[t0p : t0p + BLK, tcn, mt * P : (mt + 1) * P],
                        ident16[t0p : t0p + BLK, t0p + ti : t0p + ti + 1],
                        start=True,
                        stop=stop,
                        skip_group_check=True,
                    )

                def emit_qk_block(b):
                    cols = slice(32 * b, 32 * b + 32)
                    qp = qk_ps.tile([P, 4, 32], F32, tag="qkps", name=f"qkps{b}")
                    for wi, w_sb in enumerate((wac, wap)):
                        for mt in range(2):
                            g = 2 * wi + mt
                            for kt in range(2):
                                nc.tensor.matmul(
                                    qp[:, g, :],
                                    w_sb[:, kt, mt * P : (mt + 1) * P],
                                    combT[:, 2 + kt, cols],
                                    start=(kt == 0),
                                    stop=(kt == 1),
                                )
                    for mt in range(2):
                        if zb:
                            nc.vector.tensor_copy(qT[:, mt, cols], qp[:, mt, :])
                            nc.vector.tensor_copy(kTb[:, mt, cols], qp[:, 2 + mt, :])
                        else:
                            nc.scalar.activation(
                                qT[:, mt, cols],
                                qp[:, mt, :],
                                AF.Identity,
                                bias=bac[:, mt : mt + 1],
                            )
                            nc.scalar.activation(
                                kTb[:, mt, cols],
                                qp[:, 2 + mt, :],
                                AF.Identity,
                                bias=bap[:, mt : mt + 1],
                            )

                # chunk pipeline state: one attention ROW per scan step.
                # Row r (global t-index): adds -> tanh -> v-reduce+copy ->
                # (per chunk) scatter. jcap is uniform per chunk of 8 rows.
                et_tiles = {}
                row_tiles = {}

                def row_jcap(r):
                    return (r // CH) * CH + CH

                def emit_row_adds(r):
                    jcap = row_jcap(r)
                    et_r = ep.tile([P, 2, 2 * P], FP16, tag="etile", name=f"et{r}")
                    et_tiles[r] = et_r
                    nc.vector.tensor_scalar_add(
                        et_r[:, 0, :jcap], kTb[:, 0, :jcap], qT[:, 0, r : r + 1]
                    )
                    nc.gpsimd.tensor_scalar_add(
                        et_r[:, 1, :jcap], kTb[:, 1, :jcap], qT[:, 1, r : r + 1]
                    )

                def emit_row_tanh(r):
                    jcap = row_jcap(r)
                    et_r = et_tiles[r]
                    nc.scalar.activation(
                        et_r[:, :, :jcap], et_r[:, :, :jcap], AF.Tanh
                    )

                def emit_row_vred(r):
                    jcap = row_jcap(r)
                    c, tl = r // CH, r % CH
                    et_r = et_tiles.pop(r)
                    if tl == 0:
                        row_tiles[c] = rp.tile(
                            [1, CH * T], F32, tag="rowtile", name=f"rw{c}"
                        )
                    psv = e_ps.tile([1, 512], F32, tag="vps", name=f"vps{r}")
                    for kt in range(2):
                        nc.tensor.matmul(
                            psv[0:1, :jcap],
                            vcol[:, kt : kt + 1],
                            et_r[:, kt, :jcap],
                            start=(kt == 0),
                            stop=(kt == 1),
                        )
                    nc.vector.tensor_copy(
                        row_tiles[c][:, tl * jcap : (tl + 1) * jcap],
                        psv[0:1, :jcap],
                    )

                def emit_scatter(c):
                    jcap = c * CH + CH
                    w = CH * jcap
                    row = row_tiles.pop(c)
                    tc_i = (c * CH) // P
                    tp0 = (c * CH) % P
                    scr = scrp.tile([CH, T], F32, tag="scr", name=f"scr{c}")
                    nc.gpsimd.dma_start(
                        scr[:, 0:jcap],
                        row[0:1, :w].rearrange("p (t j) -> p t j", j=jcap),
                    )
                    nc.gpsimd.dma_start(
                        scores[tp0 : tp0 + CH, tc_i, 0:jcap], scr[:, 0:jcap]
                    )

                def emit_row_stage(g):
                    """Pipeline stages for global step-index g (one call/step)."""
                    if 0 <= g < P:
                        emit_row_adds(g)
                    if 0 <= g - 1 < P:
                        emit_row_tanh(g - 1)
                    if 0 <= g - 2 < P:
                        emit_row_vred(g - 2)
                        if (g - 2) % CH == CH - 1:
                            emit_scatter((g - 2) // CH)

                def emit_hs_half(tc_i):
                    for ht in range(2):
                        tr_ps = qk_ps.tile(
                            [P, P], FP16, tag="qkps", name=f"hst{tc_i}{ht}"
                        )
                        nc.tensor.transpose(
                            tr_ps[:],
                            combT[:, 2 + ht, tc_i * P : (tc_i + 1) * P],
                            ident16[:],
                        )
                        nc.vector.tensor_copy(
                            hs[:, tc_i, ht * P : (ht + 1) * P], tr_ps[:]
                        )

                def emit_softmax_half(tc_i):
                    sl = scores[:, tc_i, :]
                    nc.vector.tensor_tensor(
                        sl, sl, maskadd[:, tc_i, :], mybir.AluOpType.add
                    )
                    nc.scalar.activation(sl, sl, AF.Exp)
                    nc.vector.reduce_sum(
                        ssum[:, tc_i : tc_i + 1], sl, axis=mybir.AxisListType.X
                    )
                    nc.vector.reciprocal(
                        srecip[:, tc_i : tc_i + 1], ssum[:, tc_i : tc_i + 1]
                    )
                    nc.vector.tensor_tensor(
                        sl,
                        sl,
                        srecip[:, tc_i : tc_i + 1].to_broadcast([P, T]),
                        mybir.AluOpType.mult,
                    )
                    for jt in range(2):
                        tr_ps = qk_ps.tile(
                            [P, P], F32, tag="qkps", name=f"atr{tc_i}{jt}"
                        )
                        nc.tensor.transpose(
                            tr_ps[:], scores[:, tc_i, jt * P : (jt + 1) * P], ident[:]
                        )
                        nc.vector.tensor_copy(
                            alphaT[:, jt, tc_i * P : (tc_i + 1) * P], tr_ps[:]
                        )

                # --- out-proj micro-emission thunks (interleaved into scan) ---
                bsl_tiles = {}
                emitted_op = set()

                def outproj_thunks(s, mt):
                    """Thunk list for one (slab, mt): 8 mms + 2 copies + dma."""
                    emitted_op.add((s, mt))
                    n0 = s * NS
                    wsl = wsl_tiles[s]
                    nsub = NS // SUB
                    osb = op.tile([P, NS], FP16, tag=f"osb{mt}", name=f"osb{s}_{mt}")
                    if not zb and s not in bsl_tiles:
                        bsl = bp.tile([1, NS], BF16, tag="bslab")
                        nc.sync.dma_start(bsl[:], bout_d[:, n0 : n0 + NS])
                        bsl_tiles[s] = bsl
                    thunks = []
                    pss_box = {}
                    for i in range(nsub):
                        def mk_mm(i, kt):
                            def f():
                                if kt == 0:
                                    pss_box[i] = pg_ps.tile(
                                        [P, SUB], F32, tag="ops", name=f"ops{s}_{mt}{i}"
                                    )
                                nc.tensor.matmul(
                                    pss_box[i][:],
                                    combTr[:, kt, mt * P : (mt + 1) * P],
                                    wsl[:, kt, i * SUB : (i + 1) * SUB],
                                    start=(kt == 0),
                                    stop=(zb and kt == 3),
                                )
                                if (not zb) and kt == 3:
                                    nc.tensor.matmul(
                                        pss_box[i][:],
                                        ones_row[:],
                                        bsl_tiles[s][:, i * SUB : (i + 1) * SUB],
                                        start=False,
                                        stop=True,
                                    )
                            return f
                        for kt in range(4):
                            thunks.append(mk_mm(i, kt))
                        def mk_copy(i):
                            def f():
                                if (s + i) % 2 == 1:
                                    nc.vector.tensor_copy(
                                        osb[:, i * SUB : (i + 1) * SUB], pss_box[i][:]
                                    )
                                else:
                                    nc.scalar.activation(
                                        osb[:, i * SUB : (i + 1) * SUB],
                                        pss_box[i][:],
                                        AF.Copy,
                                    )
                            return f
                        thunks.append(mk_copy(i))
                    def mk_dma():
                        def f():
                            nc.sync.dma_start(
                                out_d[mt * P : (mt + 1) * P, n0 : n0 + NS], osb[:]
                            )
                        return f
                    thunks.append(mk_dma())
                    return thunks

                inscan_op = []  # queue of thunks to drain during late scan

                def emit_ctx_half(tc_i):
                    """context^T for t-half tc_i -> combT[:,0:2] + combTr copy."""
                    for mt in range(2):
                        cps = qk_ps.tile([P, P], F32, tag="qkps", name=f"ctx{tc_i}{mt}")
                        for jt in range(2):
                            nc.tensor.matmul(
                                cps[:],
                                hs[:, jt, mt * P : (mt + 1) * P],
                                alphaT[:, jt, tc_i * P : (tc_i + 1) * P],
                                start=(jt == 0),
                                stop=(jt == 1),
                            )
                        nc.scalar.activation(
                            combT[:, mt, tc_i * P : (tc_i + 1) * P], cps[:], AF.Copy
                        )
                    if tc_i == 0:
                        # t=0 has no past: zero the context column
                        nc.gpsimd.memset(combT[:, 0:2, 0:1], 0.0)
                    nc.vector.tensor_copy(
                        combTr[:, :, tc_i * P : (tc_i + 1) * P],
                        combT[:, :, tc_i * P : (tc_i + 1) * P],
                    )

                nc.gpsimd.memset(scores[:], 0.0)
                ps0 = sc_ps.tile([P, 2], F32, tag="scanstep", name="sc0")
                emit_xmm(ps0, 0, 0, stop=True)
                emit_xmm(ps0, 0, 1, stop=True)
                nc.scalar.activation(combT[:, 2:4, 0:1], ps0[:, :], AF.Tanh)
                for t in range(1, T):
                    ps_t = sc_ps.tile([P, 2], F32, tag="scanstep", name=f"sc{t}")
                    for mt in range(2):
                        emit_xmm(ps_t, t, mt)
                        for kt in range(2):
                            nc.tensor.matmul(
                                ps_t[:, mt : mt + 1],
                                whh[:, kt, mt * P : (mt + 1) * P],
                                combT[:, 2 + kt, t - 1 : t],
                                start=False,
                                stop=(kt == 1),
                                skip_group_check=True,
                            )
                    nc.scalar.activation(
                        combT[:, 2:4, t : t + 1], ps_t[:, :], AF.Tanh
                    )
                    if t % 32 == 31:
                        emit_qk_block(t // 32)
                    if t >= 39:
                        emit_row_stage(t - 39)
                    if t == 127:
                        emit_hs_half(0)
                    if t == 175:
                        emit_softmax_half(0)
                    if t == 183:
                        emit_ctx_half(0)
                emit_hs_half(1)
                # tail: rows 128..255 (late chunks) interleaved with the
                # mt=0 out-proj (ready since ctx0) to keep all engines busy.
                for s_ in range(NPRE):
                    inscan_op.extend(outproj_thunks(s_, 0))

                def drain_some(k):
                    for _ in range(min(k, len(inscan_op))):
                        inscan_op.pop(0)()

                for g in range(P, T + 3):
                    if g < T:
                        emit_row_adds(g)
                    if P <= g - 1 < T:
                        emit_row_tanh(g - 1)
                    if P <= g - 2 < T:
                        emit_row_vred(g - 2)
                        if (g - 2) % CH == CH - 1:
                            emit_scatter((g - 2) // CH)
                    drain_some(1)
                emit_softmax_half(1)
                drain_some(len(inscan_op))
                emit_ctx_half(1)
                if dbg:
                    nc.sync.dma_start(dbg_scores[:], scores[:])
                    nc.gpsimd.dma_start(dbg_comb[:], combT[:])

            # --- phase G: remaining out-proj work ---
            # (slab s, mt=0) leads; (slab s, mt=1) trails by MT_LAG slabs so
            # the mt=1 inputs (late attention) have time to finish.
            n_slabs = V // NS

            def run_outproj(s, mt):
                if (s, mt) in emitted_op:
                    return
                for th in outproj_thunks(s, mt):
                    th()

            for s in range(n_slabs):
                if s not in wsl_tiles:
                    wsl = wp.tile([P, 4, NS], BF16, tag="wslab")
                    nc.sync.dma_start(
                        wsl[:], wout_d[:, :, s * NS : (s + 1) * NS]
                    )
                    wsl_tiles[s] = wsl
                run_outproj(s, 0)
                if s >= MT_LAG:
                    run_outproj(s - MT_LAG, 1)
            for s in range(n_slabs - MT_LAG, n_slabs):
                run_outproj(s, 1)
            bp_cm.__exit__(None, None, None)
            op_cm.__exit__(None, None, None)
            pg_cm.__exit__(None, None, None)
            wp_cm.__exit__(None, None, None)

    nc.compile()
    return nc


_NC_CACHE = {}


def _get_nc(zb, dbg=False):
    key = ("nc", zb, dbg)
    if key not in _NC_CACHE:
        _NC_CACHE[key] = build_nc(zb=zb, dbg=dbg)
    return _NC_CACHE[key]


def _prep(inputs):
    input = np.asarray(inputs["input"])
    embedding = np.ascontiguousarray(np.asarray(inputs["embedding"], np.float32))
    W_ih, b_ih = inputs["W_ih"], inputs["b_ih"]
    W_hh, b_hh = inputs["W_hh"], inputs["b_hh"]
    W_ac, b_ac = inputs["W_ac"], inputs["b_ac"]
    W_ap, b_ap = inputs["W_ap"], inputs["b_ap"]
    v_attn, W_out, b_out = inputs["v_attn"], inputs["W_out"], inputs["b_out"]
    zb = bool(
        not np.any(b_ih)
        and not np.any(b_hh)
        and not np.any(b_ac)
        and not np.any(b_ap)
        and not np.any(b_out)
    )

    t_idx = np.arange(T)
    j_idx = np.arange(T)
    maskadd = np.where(
        j_idx[None, :] < (t_idx[:, None]), 0.0, -1e9
    ).astype(np.float32)  # [t, j]
    maskadd = np.ascontiguousarray(
        maskadd.reshape(2, P, T).transpose(1, 0, 2)
    )  # [tp, tc, j]

    import ml_dtypes

    wout_r = np.ascontiguousarray(
        np.asarray(W_out, np.float32)
        .astype(ml_dtypes.bfloat16)
        .reshape(4, P, V)
        .transpose(1, 0, 2)
    )
    bxrow = (np.asarray(b_ih, np.float32) + np.asarray(b_hh, np.float32)).reshape(1, H)
    bxrow = np.ascontiguousarray(
        np.concatenate([bxrow, np.zeros((1, H), np.float32)], axis=1)
    )
    shared = {
        "emb": embedding,
        "wih": _r2(np.asarray(W_ih, np.float32)),
        "whh": _r2(np.asarray(W_hh, np.float32).astype(SCAN_NP)),
        "wac": _r2(np.asarray(W_ac, np.float32).astype(np.float16)),
        "wap": _r2(np.asarray(W_ap, np.float32).astype(np.float16)),
        "bxrow": bxrow,
        "bac": _col(np.asarray(b_ac, np.float32)),
        "bap": _col(np.asarray(b_ap, np.float32)),
        "vcol": _col(np.asarray(v_attn, np.float32).astype(np.float16)),
        "maskadd": maskadd,
        "wout": wout_r,
        "bout": np.ascontiguousarray(
            np.asarray(b_out, np.float32).astype(ml_dtypes.bfloat16)[None, :]
        ),
        "ones": np.ones((1, P), ml_dtypes.bfloat16),
    }
    in_maps = []
    for b in range(B):
        m = dict(shared)
        m["idx"] = np.ascontiguousarray(
            input[b].reshape(2, P).T.astype(np.int32)
        )
        in_maps.append(m)

    return in_maps, zb


def _run(inputs, trace=False, dbg=False):
    in_maps, zb = _prep(inputs)
    nc = _get_nc(zb, dbg=dbg)
    res = run_bass_kernel_spmd(nc, in_maps, list(range(NCORE)), trace=trace)
    out = np.stack([res.results[c]["out"] for c in range(NCORE)], axis=0)
    if dbg:
        extras = {
            k: np.stack([res.results[c][k] for c in range(NCORE)], axis=0)
            for k in ("dbg_scores", "dbg_comb", "dbg_xrow")
        }
        return np.ascontiguousarray(out.astype(np.float32)), res.exec_time_ns, extras
    return np.ascontiguousarray(out.astype(np.float32)), res.exec_time_ns


def kernel(**inputs):
    return _run(inputs)[0]


# revision 23
# speedup vs baseline: 1.8238x; 1.8238x over previous
"""AttentionRNN Trainium2 kernel.

Data-parallel across 8 NeuronCores on the batch axis (B=8 -> 1 sequence per
core). Everything (embedding gather, input projection, sequential RNN scan,
additive attention, output projection) runs on-device; the host only reshapes
and shards inputs and reassembles the output.

v2: single-ACT scan step. x_proj is pre-folded into the scan PSUM block via a
matmul (32 steps per PSUM block), so each scan step is 4 tiny whh matmuls +
one [128,2] tanh. Attention energy is emitted in 2-row pieces spread across
scan steps so no single ACT instruction stalls the scan chain. The output
projection interleaves mt=0 (ready early) ahead of mt=1.

Self-contained: hardcodes all shapes; reads nothing from disk.
"""

import sys

sys.path.insert(0, "/opt/trn_rl_repo")

import numpy as np

import concourse.bacc as bacc
import concourse.mybir as mybir
import concourse.tile as tile
from concourse.bass import IndirectOffsetOnAxis
from concourse.bass_utils import run_bass_kernel_spmd
from concourse.masks import make_identity

V, E, H, B, T = 32000, 256, 256, 8, 256
P = 128
NCORE = 8
F32 = mybir.dt.float32
I32 = mybir.dt.int32
AF = mybir.ActivationFunctionType
BF16 = mybir.dt.bfloat16
FP16 = mybir.dt.float16
SCAN_NP = np.float16  # numpy dtype fed to the whh input
CH = 8  # t-chunk size in the attention energy loop
NS = 1000  # free-dim slab for the output projection stream
SUB = 500  # matmul free-dim sub-chunk (one PSUM bank)
NPRE = 15  # W_out slabs prefetched from kernel start
BLK = 64  # scan PSUM block (steps per x-prefold)
MT_LAG = 2  # how many slabs mt=1 trails mt=0 in the out-proj


def _r2(w):
    """[2P, M] -> [P, 2, M] with w2[p, k, m] = w[k*P+p, m]"""
    return np.ascontiguousarray(w.reshape(2, P, -1).transpose(1, 0, 2))


def _col(b):
    """[2P] -> [P, 2] with c[p, k] = b[k*P+p]"""
    return np.ascontiguousarray(b.reshape(2, P).T)


def build_nc(dbg=False, zb=False):
    nc = bacc.Bacc("TRN2", target_bir_lowering=False, debug=False)

    idx_d = nc.dram_tensor("idx", [P, 2], I32, kind="ExternalInput")
    emb_d = nc.dram_tensor("emb", [V, E], F32, kind="ExternalInput")
    wih_d = nc.dram_tensor("wih", [P, 2, H], F32, kind="ExternalInput")
    whh_d = nc.dram_tensor("whh", [P, 2, H], FP16, kind="ExternalInput")
    wac_d = nc.dram_tensor("wac", [P, 2, H], FP16, kind="ExternalInput")
    wap_d = nc.dram_tensor("wap", [P, 2, H], FP16, kind="ExternalInput")
    bxrow_d = nc.dram_tensor("bxrow", [1, 2 * H], F32, kind="ExternalInput")
    bac_d = nc.dram_tensor("bac", [P, 2], F32, kind="ExternalInput")
    bap_d = nc.dram_tensor("bap", [P, 2], F32, kind="ExternalInput")
    v_d = nc.dram_tensor("vcol", [P, 2], FP16, kind="ExternalInput")
    mask_d = nc.dram_tensor("maskadd", [P, 2, T], F32, kind="ExternalInput")
    wout_d = nc.dram_tensor("wout", [P, 4, V], BF16, kind="ExternalInput")
    bout_d = nc.dram_tensor("bout", [1, V], BF16, kind="ExternalInput")
    ones_d = nc.dram_tensor("ones", [1, P], BF16, kind="ExternalInput")
    out_d = nc.dram_tensor("out", [T, V], FP16, kind="ExternalOutput")
    if dbg:
        dbg_scores = nc.dram_tensor("dbg_scores", [P, 2, T], F32, kind="ExternalOutput")
        dbg_comb = nc.dram_tensor("dbg_comb", [P, 4, T], FP16, kind="ExternalOutput")
        dbg_xrow = nc.dram_tensor("dbg_xrow", [P, 2, H], FP16, kind="ExternalOutput")

    with tile.TileContext(nc) as tc:
        with tc.tile_pool(name="persist", bufs=1) as pp:
            # --- persistent SBUF state ---
            idx_sb = pp.tile([P, 2], I32)
            wih = pp.tile([P, 2, H], F32)
            whh = pp.tile([P, 2, H], FP16)
            wac = pp.tile([P, 2, H], FP16)
            wap = pp.tile([P, 2, H], FP16)
            bxrow = pp.tile([1, 2 * H], F32)
            bac = pp.tile([P, 2], F32)
            bap = pp.tile([P, 2], F32)
            vcol = pp.tile([P, 2], FP16)
            maskadd = pp.tile([P, 2, T], F32)
            ident = pp.tile([P, P], F32)
            ident16 = pp.tile([P, P], FP16)
            ones_row = pp.tile([1, P], BF16)
            ones32 = pp.tile([1, P], F32)
            embT = pp.tile([P, 2, T], F32)  # [e_p, et, t]
            xrow = pp.tile([P, 2, H], FP16)  # [t_p, tc, h] = x_proj rows
            combT = pp.tile([P, 4, T], FP16)  # [:,0:2]=context^T, [:,2:4]=Hs^T
            qT = pp.tile([P, 2, T], F32)
            kTb = pp.tile([P, 2, T], FP16)  # fp16 copy of kT for the energy adds
            scores = pp.tile([P, 2, T], F32)  # [tp, tc, j], t = tc*128+tp
            ssum = pp.tile([P, 2], F32)
            srecip = pp.tile([P, 2], F32)
            alphaT = pp.tile([P, 2, T], FP16)  # [j_p, jt, t]
            hs = pp.tile([P, 2, H], FP16)  # [t_p, tc, h] (Hs, untransposed)
            combTr = pp.tile([P, 4, T], BF16)  # bf16 copy for the out-proj

            nc.sync.dma_start(idx_sb[:], idx_d[:])
            nc.sync.dma_start(wih[:], wih_d[:])
            nc.sync.dma_start(whh[:], whh_d[:])
            nc.sync.dma_start(wac[:], wac_d[:])
            nc.sync.dma_start(wap[:], wap_d[:])
            nc.sync.dma_start(bxrow[:], bxrow_d[:])
            nc.sync.dma_start(bac[:], bac_d[:])
            nc.sync.dma_start(bap[:], bap_d[:])
            nc.sync.dma_start(vcol[:], v_d[:])
            nc.sync.dma_start(maskadd[:], mask_d[:])
            make_identity(nc, ident[:])
            nc.vector.tensor_copy(ident16[:], ident[:])
            nc.sync.dma_start(ones_row[:], ones_d[:])
            nc.gpsimd.memset(ones32[:], 1.0)
            nc.gpsimd.memset(hs[:], 0.0)

            # W_out slab prefetch pool: created first so slab DMAs stream
            # during the scan/attention phases.
            wp_cm = tc.tile_pool(name="wpool", bufs=NPRE)
            wp = wp_cm.__enter__()
            wsl_tiles = {}
            for s_ in range(NPRE):
                wsl = wp.tile([P, 4, NS], BF16, tag="wslab")
                nc.sync.dma_start(wsl[:], wout_d[:, :, s_ * NS : (s_ + 1) * NS])
                wsl_tiles[s_] = wsl

            # out-proj PSUM pool opened early so its 2 banks are reserved
            # (no aliasing stalls at the phase boundary).
            pg_cm = tc.tile_pool(name="pg_ps", bufs=2, space="PSUM")
            pg_ps = pg_cm.__enter__()
            op_cm = tc.tile_pool(name="opool", bufs=3)
            op = op_cm.__enter__()
            bp_cm = tc.tile_pool(name="bpool", bufs=2)
            bp = bp_cm.__enter__()

            # --- phase A: embedding gather + transpose to embT [e, t] ---
            with (
                tc.tile_pool(name="pha", bufs=2) as pa,
                tc.tile_pool(name="pha_ps", bufs=2, space="PSUM") as pa_ps,
            ):
                for c in range(2):
                    emb_g = pa.tile([P, E], F32, tag="embg")
                    nc.gpsimd.indirect_dma_start(
                        out=emb_g[:],
                        out_offset=None,
                        in_=emb_d[:, :],
                        in_offset=IndirectOffsetOnAxis(ap=idx_sb[:, c : c + 1], axis=0),
                    )
                    for et in range(2):
                        tr_ps = pa_ps.tile([P, P], F32, tag="trps")
                        nc.tensor.transpose(
                            tr_ps[:], emb_g[:, et * P : (et + 1) * P], ident[:]
                        )
                        nc.vector.tensor_copy(
                            embT[:, et, c * P : (c + 1) * P], tr_ps[:]
                        )

                # --- phase B: xrow[t, h] = emb @ W_ih (+ b_ih + b_hh) ---
                for tcn in range(2):
                    psx = pa_ps.tile([P, H], F32, tag="projps")
                    for et in range(2):
                        nc.tensor.matmul(
                            psx[:],
                            embT[:, et, tcn * P : (tcn + 1) * P],
                            wih[:, et, :],
                            start=(et == 0),
                            stop=False,
                            skip_group_check=True,
                        )
                    nc.tensor.matmul(
                        psx[:],
                        ones32[:],
                        bxrow[:, 0:H],
                        start=False,
                        stop=True,
                        skip_group_check=True,
                    )
                    nc.vector.tensor_copy(xrow[:, tcn, :], psx[:])
                if dbg:
                    nc.sync.dma_start(dbg_xrow[:], xrow[:])

            # --- phases C+D+E fused: scan with attention pipelined under it ---
            n_chunks = T // CH
            with (
                tc.tile_pool(name="scan_ps", bufs=2, space="PSUM") as sc_ps,
                tc.tile_pool(name="qk_ps", bufs=2, space="PSUM") as qk_ps,
                tc.tile_pool(name="epool", bufs=4) as ep,
                tc.tile_pool(name="rowpool", bufs=2) as rp,
                tc.tile_pool(name="scrpool", bufs=4, space="DRAM") as scrp,
                tc.tile_pool(name="eps", bufs=2, space="PSUM") as e_ps,
            ):
                def emit_xmm(ps_t, t, mt, stop=False):
                    """x_t into PSUM column (start=True) — resets the column.
                    MUST be immediately followed (in program order) by that
                    same (mt, column)'s whh matmuls: PSUM accumulation groups
                    are per-bank, and a later start=True begins a new group
                    whose first write to an address OVERWRITES it."""
                    t0 = (t // BLK) * BLK
                    tcn, t0p = t0 // P, t0 % P
                    ti = t - t0
                    nc.tensor.matmul(
                        ps_t[:, mt : mt + 1],
                        xrow[t0p : t0p + BLK, tcn, mt * P : (mt + 1) * P],
                        ident16[t0p : t0p + BLK, t0p + ti : t0p + ti + 1],
                        start=True,
                        stop=stop,
                        skip_group_check=True,
                    )

                def emit_qk_block(b):
                    cols = slice(32 * b, 32 * b + 32)
                    qp = qk_ps.tile([P, 4, 32], F32, tag="qkps", name=f"qkps{b}")
                    for wi, w_sb in enumerate((wac, wap)):
                        for mt in range(2):
                            g = 2 * wi + mt
                            for kt in range(2):
                                nc.tensor.matmul(
                                    qp[:, g, :],
                                    w_sb[:, kt, mt * P : (mt + 1) * P],
                                    combT[:, 2 + kt, cols],
                                    start=(kt == 0),
                                    stop=(kt == 1),
                                )
                    for mt in range(2):
                        if zb:
                            nc.vector.tensor_copy(qT[:, mt, cols], qp[:, mt, :])
                            nc.vector.tensor_copy(kTb[:, mt, cols], qp[:, 2 + mt, :])
                        else:
                            nc.scalar.activation(
                                qT[:, mt, cols],
                                qp[:, mt, :],
                                AF.Identity,
                                bias=bac[:, mt : mt + 1],
                            )
                            nc.scalar.activation(
                                kTb[:, mt, cols],
                                qp[:, 2 + mt, :],
                                AF.Identity,
                                bias=bap[:, mt : mt + 1],
                            )

                # chunk pipeline state: one attention ROW per scan step.
                # Row r (global t-index): adds -> tanh -> v-reduce+copy ->
                # (per chunk) scatter. jcap is uniform per chunk of 8 rows.
                et_tiles = {}
                row_tiles = {}

                def row_jcap(r):
                    return (r // CH) * CH + CH

                def emit_row_adds(r):
                    jcap = row_jcap(r)
                    et_r = ep.tile([P, 2, 2 * P], FP16, tag="etile", name=f"et{r}")
                    et_tiles[r] = et_r
                    for kt in range(2):
                        nc.vector.tensor_scalar_add(
                            et_r[:, kt, :jcap], kTb[:, kt, :jcap], qT[:, kt, r : r + 1]
                        )

                def emit_row_tanh(r):
                    jcap = row_jcap(r)
                    et_r = et_tiles[r]
                    nc.scalar.activation(
                        et_r[:, :, :jcap], et_r[:, :, :jcap], AF.Tanh
                    )

                def emit_row_vred(r):
                    jcap = row_jcap(r)
                    c, tl = r // CH, r % CH
                    et_r = et_tiles.pop(r)
                    if tl == 0:
                        row_tiles[c] = rp.tile(
                            [1, CH * T], F32, tag="rowtile", name=f"rw{c}"
                        )
                    psv = e_ps.tile([1, 512], F32, tag="vps", name=f"vps{r}")
                    for kt in range(2):
                        nc.tensor.matmul(
                            psv[0:1, :jcap],
                            vcol[:, kt : kt + 1],
                            et_r[:, kt, :jcap],
                            start=(kt == 0),
                            stop=(kt == 1),
                        )
                    nc.vector.tensor_copy(
                        row_tiles[c][:, tl * jcap : (tl + 1) * jcap],
                        psv[0:1, :jcap],
                    )

                def emit_scatter(c):
                    jcap = c * CH + CH
                    w = CH * jcap
                    row = row_tiles.pop(c)
                    tc_i = (c * CH) // P
                    tp0 = (c * CH) % P
                    scr = scrp.tile([CH, T], F32, tag="scr", name=f"scr{c}")
                    nc.gpsimd.dma_start(
                        scr[:, 0:jcap],
                        row[0:1, :w].rearrange("p (t j) -> p t j", j=jcap),
                    )
                    nc.gpsimd.dma_start(
                        scores[tp0 : tp0 + CH, tc_i, 0:jcap], scr[:, 0:jcap]
                    )

                def emit_row_stage(g):
                    """Pipeline stages for global step-index g (one call/step)."""
                    if 0 <= g < P:
                        emit_row_adds(g)
                    if 0 <= g - 1 < P:
                        emit_row_tanh(g - 1)
                    if 0 <= g - 2 < P:
                        emit_row_vred(g - 2)
                        if (g - 2) % CH == CH - 1:
                            emit_scatter((g - 2) // CH)

                def emit_hs_half(tc_i):
                    for ht in range(2):
                        tr_ps = qk_ps.tile(
                            [P, P], FP16, tag="qkps", name=f"hst{tc_i}{ht}"
                        )
                        nc.tensor.transpose(
                            tr_ps[:],
                            combT[:, 2 + ht, tc_i * P : (tc_i + 1) * P],
                            ident16[:],
                        )
                        nc.vector.tensor_copy(
                            hs[:, tc_i, ht * P : (ht + 1) * P], tr_ps[:]
                        )

                def emit_softmax_half(tc_i):
                    sl = scores[:, tc_i, :]
                    nc.vector.tensor_tensor(
                        sl, sl, maskadd[:, tc_i, :], mybir.AluOpType.add
                    )
                    nc.scalar.activation(sl, sl, AF.Exp)
                    nc.vector.reduce_sum(
                        ssum[:, tc_i : tc_i + 1], sl, axis=mybir.AxisListType.X
                    )
                    nc.vector.reciprocal(
                        srecip[:, tc_i : tc_i + 1], ssum[:, tc_i : tc_i + 1]
                    )
                    nc.vector.tensor_tensor(
                        sl,
                        sl,
                        srecip[:, tc_i : tc_i + 1].to_broadcast([P, T]),
                        mybir.AluOpType.mult,
                    )
                    for jt in range(2):
                        tr_ps = qk_ps.tile(
                            [P, P], F32, tag="qkps", name=f"atr{tc_i}{jt}"
                        )
                        nc.tensor.transpose(
                            tr_ps[:], scores[:, tc_i, jt * P : (jt + 1) * P], ident[:]
                        )
                        nc.vector.tensor_copy(
                            alphaT[:, jt, tc_i * P : (tc_i + 1) * P], tr_ps[:]
                        )

                # --- out-proj micro-emission thunks (interleaved into scan) ---
                bsl_tiles = {}
                emitted_op = set()

                def outproj_thunks(s, mt):
                    """Thunk list for one (slab, mt): 8 mms + 2 copies + dma."""
                    emitted_op.add((s, mt))
                    n0 = s * NS
                    wsl = wsl_tiles[s]
                    nsub = NS // SUB
                    osb = op.tile([P, NS], FP16, tag=f"osb{mt}", name=f"osb{s}_{mt}")
                    if not zb and s not in bsl_tiles:
                        bsl = bp.tile([1, NS], BF16, tag="bslab")
                        nc.sync.dma_start(bsl[:], bout_d[:, n0 : n0 + NS])
                        bsl_tiles[s] = bsl
                    thunks = []
                    pss_box = {}
                    for i in range(nsub):
                        def mk_mm(i, kt):
                            def f():
                                if kt == 0:
                                    pss_box[i] = pg_ps.tile(
                                        [P, SUB], F32, tag="ops", name=f"ops{s}_{mt}{i}"
                                    )
                                nc.tensor.matmul(
                                    pss_box[i][:],
                                    combTr[:, kt, mt * P : (mt + 1) * P],
                                    wsl[:, kt, i * SUB : (i + 1) * SUB],
                                    start=(kt == 0),
                                    stop=(zb and kt == 3),
                                )
                                if (not zb) and kt == 3:
                                    nc.tensor.matmul(
                                        pss_box[i][:],
                                        ones_row[:],
                                        bsl_tiles[s][:, i * SUB : (i + 1) * SUB],
                                        start=False,
                                        stop=True,
                                    )
                            return f
                        for kt in range(4):
                            thunks.append(mk_mm(i, kt))
                        def mk_copy(i):
                            def f():
                                if (s + i) % 2 == 1:
                                    nc.vector.tensor_copy(
                                        osb[:, i * SUB : (i + 1) * SUB], pss_box[i][:]
                                    )
                                else:
                                    nc.scalar.activation(
                                        osb[:, i * SUB : (i + 1) * SUB],
                                        pss_box[i][:],
                                        AF.Copy,
                                    )
                            return f
                        thunks.append(mk_copy(i))
                    def mk_dma():
                        def f():
                            nc.sync.dma_start(
                                out_d[mt * P : (mt + 1) * P, n0 : n0 + NS], osb[:]
                            )
                        return f
                    thunks.append(mk_dma())
                    return thunks

                inscan_op = []  # queue of thunks to drain during late scan

                def emit_ctx_half(tc_i):
                    """context^T for t-half tc_i -> combT[:,0:2] + combTr copy."""
                    for mt in range(2):
                        cps = qk_ps.tile([P, P], F32, tag="qkps", name=f"ctx{tc_i}{mt}")
                        for jt in range(2):
                            nc.tensor.matmul(
                                cps[:],
                                hs[:, jt, mt * P : (mt + 1) * P],
                                alphaT[:, jt, tc_i * P : (tc_i + 1) * P],
                                start=(jt == 0),
                                stop=(jt == 1),
                            )
                        nc.scalar.activation(
                            combT[:, mt, tc_i * P : (tc_i + 1) * P], cps[:], AF.Copy
                        )
                    if tc_i == 0:
                        # t=0 has no past: zero the context column
                        nc.gpsimd.memset(combT[:, 0:2, 0:1], 0.0)
                    nc.vector.tensor_copy(
                        combTr[:, :, tc_i * P : (tc_i + 1) * P],
                        combT[:, :, tc_i * P : (tc_i + 1) * P],
                    )

                nc.gpsimd.memset(scores[:], 0.0)
                ps0 = sc_ps.tile([P, 2], F32, tag="scanstep", name="sc0")
                emit_xmm(ps0, 0, 0, stop=True)
                emit_xmm(ps0, 0, 1, stop=True)
                nc.scalar.activation(combT[:, 2:4, 0:1], ps0[:, :], AF.Tanh)
                for t in range(1, T):
                    ps_t = sc_ps.tile([P, 2], F32, tag="scanstep", name=f"sc{t}")
                    for mt in range(2):
                        emit_xmm(ps_t, t, mt)
                        for kt in range(2):
                            nc.tensor.matmul(
                                ps_t[:, mt : mt + 1],
                                whh[:, kt, mt * P : (mt + 1) * P],
                                combT[:, 2 + kt, t - 1 : t],
                                start=False,
                                stop=(kt == 1),
                                skip_group_check=True,
                            )
                    nc.scalar.activation(
                        combT[:, 2:4, t : t + 1], ps_t[:, :], AF.Tanh
                    )
                    if t % 32 == 31:
                        emit_qk_block(t // 32)
                    if t >= 39:
                        emit_row_stage(t - 39)
                    if t == 127:
                        emit_hs_half(0)
                    if t == 175:
                        emit_softmax_half(0)
                    if t == 183:
                        emit_ctx_half(0)
                emit_hs_half(1)
                # tail: rows 128..255 (late chunks) interleaved with the
                # mt=0 out-proj (ready since ctx0) to keep all engines busy.
                for s_ in range(NPRE):
                    inscan_op.extend(outproj_thunks(s_, 0))

                def drain_some(k):
                    for _ in range(min(k, len(inscan_op))):
                        inscan_op.pop(0)()

                for g in range(P, T + 3):
                    if g < T:
                        emit_row_adds(g)
                    if P <= g - 1 < T:
                        emit_row_tanh(g - 1)
                    if P <= g - 2 < T:
                        emit_row_vred(g - 2)
                        if (g - 2) % CH == CH - 1:
                            emit_scatter((g - 2) // CH)
                    drain_some(1)
                emit_softmax_half(1)
                drain_some(len(inscan_op))
                emit_ctx_half(1)
                if dbg:
                    nc.sync.dma_start(dbg_scores[:], scores[:])
                    nc.gpsimd.dma_start(dbg_comb[:], combT[:])

            # --- phase G: remaining out-proj work ---
            # (slab s, mt=0) leads; (slab s, mt=1) trails by MT_LAG slabs so
            # the mt=1 inputs (late attention) have time to finish.
            n_slabs = V // NS

            def run_outproj(s, mt):
                if (s, mt) in emitted_op:
                    return
                for th in outproj_thunks(s, mt):
                    th()

            for s in range(n_slabs):
                if s not in wsl_tiles:
                    wsl = wp.tile([P, 4, NS], BF16, tag="wslab")
                    nc.sync.dma_start(
                        wsl[:], wout_d[:, :, s * NS : (s + 1) * NS]
                    )
                    wsl_tiles[s] = wsl
                run_outproj(s, 0)
                if s >= MT_LAG:
                    run_outproj(s - MT_LAG, 1)
            for s in range(n_slabs - MT_LAG, n_slabs):
                run_outproj(s, 1)
            bp_cm.__exit__(None, None, None)
            op_cm.__exit__(None, None, None)
            pg_cm.__exit__(None, None, None)
            wp_cm.__exit__(None, None, None)

    nc.compile()
    return nc


_NC_CACHE = {}


def _get_nc(zb, dbg=False):
    key = ("nc", zb, dbg)
    if key not in _NC_CACHE:
        _NC_CACHE[key] = build_nc(zb=zb, dbg=dbg)
    return _NC_CACHE[key]


def _prep(inputs):
    input = np.asarray(inputs["input"])
    embedding = np.ascontiguousarray(np.asarray(inputs["embedding"], np.float32))
    W_ih, b_ih = inputs["W_ih"], inputs["b_ih"]
    W_hh, b_hh = inputs["W_hh"], inputs["b_hh"]
    W_ac, b_ac = inputs["W_ac"], inputs["b_ac"]
    W_ap, b_ap = inputs["W_ap"], inputs["b_ap"]
    v_attn, W_out, b_out = inputs["v_attn"], inputs["W_out"], inputs["b_out"]
    zb = bool(
        not np.any(b_ih)
        and not np.any(b_hh)
        and not np.any(b_ac)
        and not np.any(b_ap)
        and not np.any(b_out)
    )

    t_idx = np.arange(T)
    j_idx = np.arange(T)
    maskadd = np.where(
        j_idx[None, :] < (t_idx[:, None]), 0.0, -1e9
    ).astype(np.float32)  # [t, j]
    maskadd = np.ascontiguousarray(
        maskadd.reshape(2, P, T).transpose(1, 0, 2)
    )  # [tp, tc, j]

    import ml_dtypes

    wout_r = np.ascontiguousarray(
        np.asarray(W_out, np.float32)
        .astype(ml_dtypes.bfloat16)
        .reshape(4, P, V)
        .transpose(1, 0, 2)
    )
    bxrow = (np.asarray(b_ih, np.float32) + np.asarray(b_hh, np.float32)).reshape(1, H)
    bxrow = np.ascontiguousarray(
        np.concatenate([bxrow, np.zeros((1, H), np.float32)], axis=1)
    )
    shared = {
        "emb": embedding,
        "wih": _r2(np.asarray(W_ih, np.float32)),
        "whh": _r2(np.asarray(W_hh, np.float32).astype(SCAN_NP)),
        "wac": _r2(np.asarray(W_ac, np.float32).astype(np.float16)),
        "wap": _r2(np.asarray(W_ap, np.float32).astype(np.float16)),
        "bxrow": bxrow,
        "bac": _col(np.asarray(b_ac, np.float32)),
        "bap": _col(np.asarray(b_ap, np.float32)),
        "vcol": _col(np.asarray(v_attn, np.float32).astype(np.float16)),
        "maskadd": maskadd,
        "wout": wout_r,
        "bout": np.ascontiguousarray(
            np.asarray(b_out, np.float32).astype(ml_dtypes.bfloat16)[None, :]
        ),
        "ones": np.ones((1, P), ml_dtypes.bfloat16),
    }
    in_maps = []
    for b in range(B):
        m = dict(shared)
        m["idx"] = np.ascontiguousarray(
            input[b].reshape(2, P).T.astype(np.int32)
        )
        in_maps.append(m)

    return in_maps, zb


def _run(inputs, trace=False, dbg=False):
    in_maps, zb = _prep(inputs)
    nc = _get_nc(zb, dbg=dbg)
    res = run_bass_kernel_spmd(nc, in_maps, list(range(NCORE)), trace=trace)
    out = np.stack([res.results[c]["out"] for c in range(NCORE)], axis=0)
    if dbg:
        extras = {
            k: np.stack([res.results[c][k] for c in range(NCORE)], axis=0)
            for k in ("dbg_scores", "dbg_comb", "dbg_xrow")
        }
        return np.ascontiguousarray(out.astype(np.float32)), res.exec_time_ns, extras
    return np.ascontiguousarray(out.astype(np.float32)), res.exec_time_ns


def kernel(**inputs):
    return _run(inputs)[0]


# revision 27
# speedup vs baseline: 1.8857x; 1.0339x over previous
"""AttentionRNN Trainium2 kernel.

Data-parallel across 8 NeuronCores on the batch axis (B=8 -> 1 sequence per
core). Everything (embedding gather, input projection, sequential RNN scan,
additive attention, output projection) runs on-device; the host only reshapes
and shards inputs and reassembles the output.

v2: single-ACT scan step. x_proj is pre-folded into the scan PSUM block via a
matmul (32 steps per PSUM block), so each scan step is 4 tiny whh matmuls +
one [128,2] tanh. Attention energy is emitted in 2-row pieces spread across
scan steps so no single ACT instruction stalls the scan chain. The output
projection interleaves mt=0 (ready early) ahead of mt=1.

Self-contained: hardcodes all shapes; reads nothing from disk.
"""

import sys

sys.path.insert(0, "/opt/trn_rl_repo")

import numpy as np

import concourse.bacc as bacc
import concourse.mybir as mybir
import concourse.tile as tile
from concourse.bass import IndirectOffsetOnAxis
from concourse.bass_utils import run_bass_kernel_spmd
from concourse.masks import make_identity

V, E, H, B, T = 32000, 256, 256, 8, 256
P = 128
NCORE = 8
F32 = mybir.dt.float32
I32 = mybir.dt.int32
AF = mybir.ActivationFunctionType
BF16 = mybir.dt.bfloat16
FP16 = mybir.dt.float16
SCAN_NP = np.float16  # numpy dtype fed to the whh input
CH = 8  # t-chunk size in the attention energy loop
NS = 1000  # free-dim slab for the output projection stream
SUB = 500  # matmul free-dim sub-chunk (one PSUM bank)
NPRE = 15  # W_out slabs prefetched from kernel start
BLK = 64  # scan PSUM block (steps per x-prefold)
MT_LAG = 2  # how many slabs mt=1 trails mt=0 in the out-proj


def _r2(w):
    """[2P, M] -> [P, 2, M] with w2[p, k, m] = w[k*P+p, m]"""
    return np.ascontiguousarray(w.reshape(2, P, -1).transpose(1, 0, 2))


def _col(b):
    """[2P] -> [P, 2] with c[p, k] = b[k*P+p]"""
    return np.ascontiguousarray(b.reshape(2, P).T)


def build_nc(dbg=False, zb=False):
    nc = bacc.Bacc("TRN2", target_bir_lowering=False, debug=False)

    idx_d = nc.dram_tensor("idx", [P, 2], I32, kind="ExternalInput")
    emb_d = nc.dram_tensor("emb", [V, E], F32, kind="ExternalInput")
    wih_d = nc.dram_tensor("wih", [P, 2, H], F32, kind="ExternalInput")
    whh_d = nc.dram_tensor("whh", [P, 2, H], FP16, kind="ExternalInput")
    wac_d = nc.dram_tensor("wac", [P, 2, H], FP16, kind="ExternalInput")
    wap_d = nc.dram_tensor("wap", [P, 2, H], FP16, kind="ExternalInput")
    bxrow_d = nc.dram_tensor("bxrow", [1, 2 * H], F32, kind="ExternalInput")
    bac_d = nc.dram_tensor("bac", [P, 2], F32, kind="ExternalInput")
    bap_d = nc.dram_tensor("bap", [P, 2], F32, kind="ExternalInput")
    v_d = nc.dram_tensor("vcol", [P, 2], FP16, kind="ExternalInput")
    mask_d = nc.dram_tensor("maskadd", [P, 2, T], F32, kind="ExternalInput")
    wout_d = nc.dram_tensor("wout", [P, 4, V], BF16, kind="ExternalInput")
    bout_d = nc.dram_tensor("bout", [1, V], BF16, kind="ExternalInput")
    ones_d = nc.dram_tensor("ones", [1, P], BF16, kind="ExternalInput")
    out_d = nc.dram_tensor("out", [T, V], FP16, kind="ExternalOutput")
    if dbg:
        dbg_scores = nc.dram_tensor("dbg_scores", [P, 2, T], F32, kind="ExternalOutput")
        dbg_comb = nc.dram_tensor("dbg_comb", [P, 4, T], FP16, kind="ExternalOutput")
        dbg_xrow = nc.dram_tensor("dbg_xrow", [P, 2, H], FP16, kind="ExternalOutput")

    with tile.TileContext(nc) as tc:
        with tc.tile_pool(name="persist", bufs=1) as pp:
            # --- persistent SBUF state ---
            idx_sb = pp.tile([P, 2], I32)
            wih = pp.tile([P, 2, H], F32)
            whh = pp.tile([P, 2, H], FP16)
            wac = pp.tile([P, 2, H], FP16)
            wap = pp.tile([P, 2, H], FP16)
            bxrow = pp.tile([1, 2 * H], F32)
            bac = pp.tile([P, 2], F32)
            bap = pp.tile([P, 2], F32)
            vcol = pp.tile([P, 2], FP16)
            maskadd = pp.tile([P, 2, T], F32)
            ident = pp.tile([P, P], F32)
            ident16 = pp.tile([P, P], FP16)
            ones_row = pp.tile([1, P], BF16)
            ones32 = pp.tile([1, P], F32)
            embT = pp.tile([P, 2, T], F32)  # [e_p, et, t]
            xrow = pp.tile([P, 2, H], FP16)  # [t_p, tc, h] = x_proj rows
            combT = pp.tile([P, 4, T], FP16)  # [:,0:2]=context^T, [:,2:4]=Hs^T
            qT = pp.tile([P, 2, T], F32)
            kTb = pp.tile([P, 2, T], FP16)  # fp16 copy of kT for the energy adds
            scores = pp.tile([P, 2, T], F32)  # [tp, tc, j], t = tc*128+tp
            ssum = pp.tile([P, 2], F32)
            srecip = pp.tile([P, 2], F32)
            alphaT = pp.tile([P, 2, T], FP16)  # [j_p, jt, t]
            hs = pp.tile([P, 2, H], FP16)  # [t_p, tc, h] (Hs, untransposed)
            combTr = pp.tile([P, 4, T], BF16)  # bf16 copy for the out-proj

            nc.sync.dma_start(idx_sb[:], idx_d[:])
            nc.sync.dma_start(wih[:], wih_d[:])
            nc.sync.dma_start(whh[:], whh_d[:])
            nc.sync.dma_start(wac[:], wac_d[:])
            nc.sync.dma_start(wap[:], wap_d[:])
            nc.sync.dma_start(bxrow[:], bxrow_d[:])
            nc.sync.dma_start(bac[:], bac_d[:])
            nc.sync.dma_start(bap[:], bap_d[:])
            nc.sync.dma_start(vcol[:], v_d[:])
            nc.sync.dma_start(maskadd[:], mask_d[:])
            make_identity(nc, ident[:])
            nc.vector.tensor_copy(ident16[:], ident[:])
            nc.sync.dma_start(ones_row[:], ones_d[:])
            nc.gpsimd.memset(ones32[:], 1.0)
            nc.gpsimd.memset(hs[:], 0.0)

            # W_out slab prefetch pool: created first so slab DMAs stream
            # during the scan/attention phases.
            wp_cm = tc.tile_pool(name="wpool", bufs=NPRE)
            wp = wp_cm.__enter__()
            wsl_tiles = {}
            for s_ in range(NPRE):
                wsl = wp.tile([P, 4, NS], BF16, tag="wslab")
                nc.sync.dma_start(wsl[:], wout_d[:, :, s_ * NS : (s_ + 1) * NS])
                wsl_tiles[s_] = wsl

            # out-proj PSUM pool opened early so its 2 banks are reserved
            # (no aliasing stalls at the phase boundary).
            pg_cm = tc.tile_pool(name="pg_ps", bufs=2, space="PSUM")
            pg_ps = pg_cm.__enter__()
            op_cm = tc.tile_pool(name="opool", bufs=3)
            op = op_cm.__enter__()
            bp_cm = tc.tile_pool(name="bpool", bufs=2)
            bp = bp_cm.__enter__()

            # --- phase A: embedding gather + transpose to embT [e, t] ---
            with (
                tc.tile_pool(name="pha", bufs=2) as pa,
                tc.tile_pool(name="pha_ps", bufs=2, space="PSUM") as pa_ps,
            ):
                for c in range(2):
                    emb_g = pa.tile([P, E], F32, tag="embg")
                    nc.gpsimd.indirect_dma_start(
                        out=emb_g[:],
                        out_offset=None,
                        in_=emb_d[:, :],
                        in_offset=IndirectOffsetOnAxis(ap=idx_sb[:, c : c + 1], axis=0),
                    )
                    for et in range(2):
                        tr_ps = pa_ps.tile([P, P], F32, tag="trps")
                        nc.tensor.transpose(
                            tr_ps[:], emb_g[:, et * P : (et + 1) * P], ident[:]
                        )
                        nc.vector.tensor_copy(
                            embT[:, et, c * P : (c + 1) * P], tr_ps[:]
                        )

                # --- phase B: xrow[t, h] = emb @ W_ih (+ b_ih + b_hh) ---
                for tcn in range(2):
                    psx = pa_ps.tile([P, H], F32, tag="projps")
                    for et in range(2):
                        nc.tensor.matmul(
                            psx[:],
                            embT[:, et, tcn * P : (tcn + 1) * P],
                            wih[:, et, :],
                            start=(et == 0),
                            stop=False,
                            skip_group_check=True,
                        )
                    nc.tensor.matmul(
                        psx[:],
                        ones32[:],
                        bxrow[:, 0:H],
                        start=False,
                        stop=True,
                        skip_group_check=True,
                    )
                    nc.vector.tensor_copy(xrow[:, tcn, :], psx[:])
                if dbg:
                    nc.sync.dma_start(dbg_xrow[:], xrow[:])

            # --- phases C+D+E fused: scan with attention pipelined under it ---
            n_chunks = T // CH
            with (
                tc.tile_pool(name="scan_ps", bufs=2, space="PSUM") as sc_ps,
                tc.tile_pool(name="qk_ps", bufs=1, space="PSUM") as qk_ps,
                tc.tile_pool(name="epool", bufs=4) as ep,
                tc.tile_pool(name="rowpool", bufs=2) as rp,
                tc.tile_pool(name="scrpool", bufs=4, space="DRAM") as scrp,
                tc.tile_pool(name="eps", bufs=1, space="PSUM") as e_ps,
            ):
                def emit_xmm(ps_t, t, mt, stop=False):
                    """x_t into PSUM column (start=True) — resets the column.
                    ps_t keeps the two mt halves in SEPARATE PSUM banks, so
                    both x-matmuls can run before the whh matmuls: PSUM
                    accumulation groups are per-bank, and within a bank a
                    later start=True begins a new group whose first write to
                    an address OVERWRITES it. K=128 (full identity column)
                    avoids PE tile-config switches between 64- and 128-row
                    matmuls."""
                    tcn, tp = t // P, t % P
                    nc.tensor.matmul(
                        ps_t[:, mt, 0:1],
                        xrow[:, tcn, mt * P : (mt + 1) * P],
                        ident16[:, tp : tp + 1],
                        start=True,
                        stop=stop,
                        skip_group_check=True,
                    )

                def emit_qk_block(b):
                    cols = slice(32 * b, 32 * b + 32)
                    qp = qk_ps.tile([P, 4, 32], F32, tag="qkps", name=f"qkps{b}")
                    for wi, w_sb in enumerate((wac, wap)):
                        for mt in range(2):
                            g = 2 * wi + mt
                            for kt in range(2):
                                nc.tensor.matmul(
                                    qp[:, g, :],
                                    w_sb[:, kt, mt * P : (mt + 1) * P],
                                    combT[:, 2 + kt, cols],
                                    start=(kt == 0),
                                    stop=(kt == 1),
                                )
                    for mt in range(2):
                        if zb:
                            nc.vector.tensor_copy(qT[:, mt, cols], qp[:, mt, :])
                            nc.vector.tensor_copy(kTb[:, mt, cols], qp[:, 2 + mt, :])
                        else:
                            nc.scalar.activation(
                                qT[:, mt, cols],
                                qp[:, mt, :],
                                AF.Identity,
                                bias=bac[:, mt : mt + 1],
                            )
                            nc.scalar.activation(
                                kTb[:, mt, cols],
                                qp[:, 2 + mt, :],
                                AF.Identity,
                                bias=bap[:, mt : mt + 1],
                            )

                # chunk pipeline state: one attention ROW per scan step.
                # Row r (global t-index): adds -> tanh -> v-reduce+copy ->
                # (per chunk) scatter. jcap is uniform per chunk of 8 rows.
                et_tiles = {}
                row_tiles = {}

                def row_jcap(r):
                    return (r // CH) * CH + CH

                def emit_row_adds(r):
                    jcap = row_jcap(r)
                    et_r = ep.tile([P, 2, 2 * P], FP16, tag="etile", name=f"et{r}")
                    et_tiles[r] = et_r
                    for kt in range(2):
                        nc.vector.tensor_scalar_add(
                            et_r[:, kt, :jcap], kTb[:, kt, :jcap], qT[:, kt, r : r + 1]
                        )

                def emit_row_tanh(r):
                    jcap = row_jcap(r)
                    et_r = et_tiles[r]
                    nc.scalar.activation(
                        et_r[:, :, :jcap], et_r[:, :, :jcap], AF.Tanh
                    )

                def emit_row_vred(r):
                    jcap = row_jcap(r)
                    c, tl = r // CH, r % CH
                    et_r = et_tiles.pop(r)
                    if tl == 0:
                        row_tiles[c] = rp.tile(
                            [1, CH * T], F32, tag="rowtile", name=f"rw{c}"
                        )
                    psv = e_ps.tile([1, 512], F32, tag="vps", name=f"vps{r}")
                    for kt in range(2):
                        nc.tensor.matmul(
                            psv[0:1, :jcap],
                            vcol[:, kt : kt + 1],
                            et_r[:, kt, :jcap],
                            start=(kt == 0),
                            stop=(kt == 1),
                        )
                    nc.vector.tensor_copy(
                        row_tiles[c][:, tl * jcap : (tl + 1) * jcap],
                        psv[0:1, :jcap],
                    )

                def emit_scatter(c):
                    jcap = c * CH + CH
                    w = CH * jcap
                    row = row_tiles.pop(c)
                    tc_i = (c * CH) // P
                    tp0 = (c * CH) % P
                    scr = scrp.tile([CH, T], F32, tag="scr", name=f"scr{c}")
                    nc.gpsimd.dma_start(
                        scr[:, 0:jcap],
                        row[0:1, :w].rearrange("p (t j) -> p t j", j=jcap),
                    )
                    nc.gpsimd.dma_start(
                        scores[tp0 : tp0 + CH, tc_i, 0:jcap], scr[:, 0:jcap]
                    )

                def emit_row_stage(g):
                    """Pipeline stages for global step-index g (one call/step)."""
                    if 0 <= g < P:
                        emit_row_adds(g)
                    if 0 <= g - 1 < P:
                        emit_row_tanh(g - 1)
                    if 0 <= g - 2 < P:
                        emit_row_vred(g - 2)
                        if (g - 2) % CH == CH - 1:
                            emit_scatter((g - 2) // CH)

                def emit_hs_half(tc_i):
                    for ht in range(2):
                        tr_ps = qk_ps.tile(
                            [P, P], FP16, tag="qkps", name=f"hst{tc_i}{ht}"
                        )
                        nc.tensor.transpose(
                            tr_ps[:],
                            combT[:, 2 + ht, tc_i * P : (tc_i + 1) * P],
                            ident16[:],
                        )
                        nc.vector.tensor_copy(
                            hs[:, tc_i, ht * P : (ht + 1) * P], tr_ps[:]
                        )

                def emit_softmax_half(tc_i):
                    sl = scores[:, tc_i, :]
                    nc.vector.tensor_tensor(
                        sl, sl, maskadd[:, tc_i, :], mybir.AluOpType.add
                    )
                    nc.scalar.activation(sl, sl, AF.Exp)
                    nc.vector.reduce_sum(
                        ssum[:, tc_i : tc_i + 1], sl, axis=mybir.AxisListType.X
                    )
                    nc.vector.reciprocal(
                        srecip[:, tc_i : tc_i + 1], ssum[:, tc_i : tc_i + 1]
                    )
                    nc.vector.tensor_tensor(
                        sl,
                        sl,
                        srecip[:, tc_i : tc_i + 1].to_broadcast([P, T]),
                        mybir.AluOpType.mult,
                    )
                    for jt in range(2):
                        tr_ps = qk_ps.tile(
                            [P, P], F32, tag="qkps", name=f"atr{tc_i}{jt}"
                        )
                        nc.tensor.transpose(
                            tr_ps[:], scores[:, tc_i, jt * P : (jt + 1) * P], ident[:]
                        )
                        nc.vector.tensor_copy(
                            alphaT[:, jt, tc_i * P : (tc_i + 1) * P], tr_ps[:]
                        )

                # --- out-proj micro-emission thunks (interleaved into scan) ---
                bsl_tiles = {}
                emitted_op = set()

                def outproj_thunks(s, mt):
                    """Thunk list for one (slab, mt): 8 mms + 2 copies + dma."""
                    emitted_op.add((s, mt))
                    n0 = s * NS
                    wsl = wsl_tiles[s]
                    nsub = NS // SUB
                    osb = op.tile([P, NS], FP16, tag=f"osb{mt}", name=f"osb{s}_{mt}")
                    if not zb and s not in bsl_tiles:
                        bsl = bp.tile([1, NS], BF16, tag="bslab")
                        nc.sync.dma_start(bsl[:], bout_d[:, n0 : n0 + NS])
                        bsl_tiles[s] = bsl
                    thunks = []
                    pss_box = {}
                    for i in range(nsub):
                        def mk_mm(i, kt):
                            def f():
                                if kt == 0:
                                    pss_box[i] = pg_ps.tile(
                                        [P, SUB], F32, tag="ops", name=f"ops{s}_{mt}{i}"
                                    )
                                nc.tensor.matmul(
                                    pss_box[i][:],
                                    combTr[:, kt, mt * P : (mt + 1) * P],
                                    wsl[:, kt, i * SUB : (i + 1) * SUB],
                                    start=(kt == 0),
                                    stop=(zb and kt == 3),
                                )
                                if (not zb) and kt == 3:
                                    nc.tensor.matmul(
                                        pss_box[i][:],
                                        ones_row[:],
                                        bsl_tiles[s][:, i * SUB : (i + 1) * SUB],
                                        start=False,
                                        stop=True,
                                    )
                            return f
                        for kt in range(4):
                            thunks.append(mk_mm(i, kt))
                        def mk_copy(i):
                            def f():
                                if (s + i) % 2 == 1:
                                    nc.vector.tensor_copy(
                                        osb[:, i * SUB : (i + 1) * SUB], pss_box[i][:]
                                    )
                                else:
                                    nc.scalar.activation(
                                        osb[:, i * SUB : (i + 1) * SUB],
                                        pss_box[i][:],
                                        AF.Copy,
                                    )
                            return f
                        thunks.append(mk_copy(i))
                    def mk_dma():
                        def f():
                            nc.sync.dma_start(
                                out_d[mt * P : (mt + 1) * P, n0 : n0 + NS], osb[:]
                            )
                        return f
                    thunks.append(mk_dma())
                    return thunks

                inscan_op = []  # queue of thunks to drain during late scan

                def emit_ctx_half(tc_i):
                    """context^T for t-half tc_i -> combT[:,0:2] + combTr copy."""
                    for mt in range(2):
                        cps = qk_ps.tile([P, P], F32, tag="qkps", name=f"ctx{tc_i}{mt}")
                        for jt in range(2):
                            nc.tensor.matmul(
                                cps[:],
                                hs[:, jt, mt * P : (mt + 1) * P],
                                alphaT[:, jt, tc_i * P : (tc_i + 1) * P],
                                start=(jt == 0),
                                stop=(jt == 1),
                            )
                        nc.scalar.activation(
                            combT[:, mt, tc_i * P : (tc_i + 1) * P], cps[:], AF.Copy
                        )
                    if tc_i == 0:
                        # t=0 has no past: zero the context column
                        nc.gpsimd.memset(combT[:, 0:2, 0:1], 0.0)
                    nc.vector.tensor_copy(
                        combTr[:, :, tc_i * P : (tc_i + 1) * P],
                        combT[:, :, tc_i * P : (tc_i + 1) * P],
                    )

                def drain_some(k):
                    for _ in range(min(k, len(inscan_op))):
                        inscan_op.pop(0)()

                nc.gpsimd.memset(scores[:], 0.0)
                ps0 = sc_ps.tile([P, 2, 512], F32, tag="scanstep", name="sc0")
                emit_xmm(ps0, 0, 0, stop=True)
                emit_xmm(ps0, 0, 1, stop=True)
                nc.scalar.activation(combT[:, 2:4, 0:1], ps0[:, :, 0:1], AF.Tanh)
                for t in range(1, T):
                    ps_t = sc_ps.tile(
                        [P, 2, 512], F32, tag="scanstep", name=f"sc{t}"
                    )
                    emit_xmm(ps_t, t, 0)
                    emit_xmm(ps_t, t, 1)
                    for mt in range(2):
                        for kt in range(2):
                            nc.tensor.matmul(
                                ps_t[:, mt, 0:1],
                                whh[:, kt, mt * P : (mt + 1) * P],
                                combT[:, 2 + kt, t - 1 : t],
                                start=False,
                                stop=(kt == 1),
                                skip_group_check=True,
                            )
                    nc.scalar.activation(
                        combT[:, 2:4, t : t + 1], ps_t[:, :, 0:1], AF.Tanh
                    )
                    if t % 32 == 31:
                        emit_qk_block(t // 32)
                    if t >= 40:
                        emit_row_stage(t - 40)
                    if t == 127:
                        emit_hs_half(0)
                    if t == 175:
                        emit_softmax_half(0)
                    if t == 183:
                        emit_ctx_half(0)
                        for s_ in range(NPRE):
                            inscan_op.extend(outproj_thunks(s_, 0))
                    if t >= 184:
                        drain_some(1)
                emit_hs_half(1)
                # tail: rows 128..255 (late chunks) interleaved with the
                # mt=0 out-proj (ready since ctx0), PE-paced via drain rate.
                for g in range(P, T + 3):
                    if g < T:
                        emit_row_adds(g)
                    if P <= g - 1 < T:
                        emit_row_tanh(g - 1)
                    if P <= g - 2 < T:
                        emit_row_vred(g - 2)
                        if (g - 2) % CH == CH - 1:
                            emit_scatter((g - 2) // CH)
                    drain_some(4)
                emit_softmax_half(1)
                drain_some(len(inscan_op))
                emit_ctx_half(1)
                if dbg:
                    nc.sync.dma_start(dbg_scores[:], scores[:])
                    nc.gpsimd.dma_start(dbg_comb[:], combT[:])

            # --- phase G: remaining out-proj work ---
            # (slab s, mt=0) leads; (slab s, mt=1) trails by MT_LAG slabs so
            # the mt=1 inputs (late attention) have time to finish.
            n_slabs = V // NS

            def run_outproj(s, mt):
                if (s, mt) in emitted_op:
                    return
                for th in outproj_thunks(s, mt):
                    th()

            for s in range(n_slabs):
                if s not in wsl_tiles:
                    wsl = wp.tile([P, 4, NS], BF16, tag="wslab")
                    nc.sync.dma_start(
                        wsl[:], wout_d[:, :, s * NS : (s + 1) * NS]
                    )
                    wsl_tiles[s] = wsl
                run_outproj(s, 0)
                if s >= MT_LAG:
                    run_outproj(s - MT_LAG, 1)
            for s in range(n_slabs - MT_LAG, n_slabs):
                run_outproj(s, 1)
            bp_cm.__exit__(None, None, None)
            op_cm.__exit__(None, None, None)
            pg_cm.__exit__(None, None, None)
            wp_cm.__exit__(None, None, None)

    nc.compile()
    return nc


_NC_CACHE = {}


def _get_nc(zb, dbg=False):
    key = ("nc", zb, dbg)
    if key not in _NC_CACHE:
        _NC_CACHE[key] = build_nc(zb=zb, dbg=dbg)
    return _NC_CACHE[key]


def _prep(inputs):
    input = np.asarray(inputs["input"])
    embedding = np.ascontiguousarray(np.asarray(inputs["embedding"], np.float32))
    W_ih, b_ih = inputs["W_ih"], inputs["b_ih"]
    W_hh, b_hh = inputs["W_hh"], inputs["b_hh"]
    W_ac, b_ac = inputs["W_ac"], inputs["b_ac"]
    W_ap, b_ap = inputs["W_ap"], inputs["b_ap"]
    v_attn, W_out, b_out = inputs["v_attn"], inputs["W_out"], inputs["b_out"]
    zb = bool(
        not np.any(b_ih)
        and not np.any(b_hh)
        and not np.any(b_ac)
        and not np.any(b_ap)
        and not np.any(b_out)
    )

    t_idx = np.arange(T)
    j_idx = np.arange(T)
    maskadd = np.where(
        j_idx[None, :] < (t_idx[:, None]), 0.0, -1e9
    ).astype(np.float32)  # [t, j]
    maskadd = np.ascontiguousarray(
        maskadd.reshape(2, P, T).transpose(1, 0, 2)
    )  # [tp, tc, j]

    import ml_dtypes

    wout_r = np.ascontiguousarray(
        np.asarray(W_out, np.float32)
        .astype(ml_dtypes.bfloat16)
        .reshape(4, P, V)
        .transpose(1, 0, 2)
    )
    bxrow = (np.asarray(b_ih, np.float32) + np.asarray(b_hh, np.float32)).reshape(1, H)
    bxrow = np.ascontiguousarray(
        np.concatenate([bxrow, np.zeros((1, H), np.float32)], axis=1)
    )
    shared = {
        "emb": embedding,
        "wih": _r2(np.asarray(W_ih, np.float32)),
        "whh": _r2(np.asarray(W_hh, np.float32).astype(SCAN_NP)),
        "wac": _r2(np.asarray(W_ac, np.float32).astype(np.float16)),
        "wap": _r2(np.asarray(W_ap, np.float32).astype(np.float16)),
        "bxrow": bxrow,
        "bac": _col(np.asarray(b_ac, np.float32)),
        "bap": _col(np.asarray(b_ap, np.float32)),
        "vcol": _col(np.asarray(v_attn, np.float32).astype(np.float16)),
        "maskadd": maskadd,
        "wout": wout_r,
        "bout": np.ascontiguousarray(
            np.asarray(b_out, np.float32).astype(ml_dtypes.bfloat16)[None, :]
        ),
        "ones": np.ones((1, P), ml_dtypes.bfloat16),
    }
    in_maps = []
    for b in range(B):
        m = dict(shared)
        m["idx"] = np.ascontiguousarray(
            input[b].reshape(2, P).T.astype(np.int32)
        )
        in_maps.append(m)

    return in_maps, zb


def _run(inputs, trace=False, dbg=False):
    in_maps, zb = _prep(inputs)
    nc = _get_nc(zb, dbg=dbg)
    res = run_bass_kernel_spmd(nc, in_maps, list(range(NCORE)), trace=trace)
    out = np.stack([res.results[c]["out"] for c in range(NCORE)], axis=0)
    if dbg:
        extras = {
            k: np.stack([res.results[c][k] for c in range(NCORE)], axis=0)
            for k in ("dbg_scores", "dbg_comb", "dbg_xrow")
        }
        return np.ascontiguousarray(out.astype(np.float32)), res.exec_time_ns, extras
    return np.ascontiguousarray(out.astype(np.float32)), res.exec_time_ns


def kernel(**inputs):
    return _run(inputs)[0]
